# revision 32
# baseline (speedup 1.0000x reference)
"""Trainium2 Bass kernel for nn_BiomechanicsLoss (masked quadratic-form loss).

Math (per point): et = [u0, v1, w2, .5(u1+v0), .5(u2+w0), .5(w1+v2)],
q = et^T C et with C = inv(compliance) cast to f32.  Loss =
sqrt(sum_masked(q^2)) / count_masked, mask = gt_sdf < 1e-8.

q = s^T A s + d*(h4^2+h5^2+h6^2) with s = (u0, v1, w2), h4 = u1+v0,
h5 = u2+w0, h6 = w1+v2, A = sym(C)[:3,:3], d = sym(C)[3,3]/4.  A is SPD,
so with Cholesky A = L L^T:  s^T A s = t1^2 + t2^2 + t3^2,
  t1 = L00 s1 + L10 s2 + L20 s3, t2 = L11 s2 + L21 s3, t3 = L22 s3.

Sharding: pure data-parallel over the N point dimension across 8 cores; each
core reduces its 524288-point shard to per-partition partials [128, 2*NT]
(per-chunk sum(mask*q^2) and count columns); the host sums partials, takes
sqrt and divides.

v2 (this file): host packs shards as bf16 (mask-safe: bf16 keeps sign and
exponent, and P(|sd| within one bf16 ulp of 1e-8) ~ 1e-10), halving HBM
traffic (21 MB -> 10.5 MB/core; the 16 DMA engines sustain ~424 GB/s
aggregate -> ~25 us streaming floor).  Compute split across DVE and ACT
(empirical: DVE tensor_tensor bf16 (F/2+151)cyc @0.96GHz, tensor_scalar
bf16 (F/4+151)cyc, scalar_tensor_tensor 1x (F+151)cyc; ACT (F+352)cyc
@1.2GHz dtype-independent):
  DVE : all three shear sums in ONE strided-AP tensor_tensor (the packed
        pair layout makes in0/in1 stride-2F views), Cholesky g/t1/t2 via
        scalar_tensor_tensor, mask+count via tensor_scalar(is_lt)+accum,
        3-op fold of the 6 squared terms, qm = q*mask.
  ACT : all 6 weighted squares (Square with pre-scale; one wide 3F op
        for the shear block, L-scales on t1/t2/w2), and the final
        Square(qm) with f32 row-accum -> per-chunk sum(q^2 m).
NOTE: nc.vector.tensor_tensor_reduce and nc.gpsimd.tensor_add pass
CoreSim but CRASH this runtime's hardware path (NRT unrecoverable) —
do not re-enable USE_TTR/USE_POOL without re-testing.
Chunks [320,512,1024,1088,1152]: ramped so DVE (the saturated pacer,
~10ns/col) never outruns the DMA feed (~6ns/col after the 8.6us
preamble) — chunk k's compute can only start once its full DMA lands,
so small-then-growing chunks start DVE at ~12us with minimal boundary
gaps.  The tail is sync/epilogue-bound (~4us fixed).  Max F=1152 is an
SBUF constraint: io bufs=4 + mid bufs=3 with the wide fold running in
place over ZA (saves the LV tile) needs 182F bytes/partition <= 208KB.
Also measured and REJECTED: tensor_reduce fold (interleaved strided ACT
writes run ~4x slow, reduce runs 1x), TS+TT pairs instead of stt
(62.3us vs 60.6us), io=5 (wash), mid=2 (62.5us), flat chunks (60.6us).
Measured: f32 baseline 78.4us -> bf16 restructure 62.5us -> in-place
fold + mid=3 60.7us -> ramped chunks 60.45us (DVE dur-sum ~43.5us with
2.7us gaps, ACT ~34us, ~8.6us to first DMA byte, ~4us tail; structural
floor with this op set ~59.8us).
"""

import numpy as np

N = 4_194_304
NCORES = 8
N_LOCAL = N // NCORES  # 524288
P = 128
J = N_LOCAL // P  # 4096 points per partition
CHUNKS = [320, 512, 1024, 1088, 1152]
NT = len(CHUNKS)
assert sum(CHUNKS) == J

THRESH = 1e-8

# exotic-op switches (bisection: stt/ttr crashed on HW despite passing
# CoreSim; the safe path uses only op/dtype combos proven in the f32
# baseline kernel)
USE_STT = True    # scalar_tensor_tensor (1x on HW but fewer instrs;
                  # measured 60.6us vs 62.3us for TS+TT pairs)
USE_TTR = False   # tensor_tensor_reduce fused mult + f32 row-accum
USE_POOL = False  # gpsimd tensor_add offload
USE_REDUCE = False  # interleaved-Z fold via one tensor_reduce(axis=X)


def _weights():
    vp, Ep = 0.4, 0.21
    Ci = np.zeros((6, 6), dtype=np.float64)
    Ci[0, 0] = 1 / Ep;  Ci[0, 1] = -vp / Ep; Ci[0, 2] = -vp / Ep
    Ci[1, 0] = -vp / Ep; Ci[1, 1] = 1 / Ep;  Ci[1, 2] = -vp / Ep
    Ci[2, 0] = -vp;      Ci[2, 1] = -vp;     Ci[2, 2] = 1 / Ep
    Ci[3, 3] = 2 * (1 + vp) / Ep
    Ci[4, 4] = Ci[3, 3]
    Ci[5, 5] = Ci[3, 3]
    # match reference: inverse computed in f64, cast to f32
    C = np.linalg.inv(Ci).astype(np.float32).astype(np.float64)
    Cs = 0.5 * (C + C.T)
    L = np.linalg.cholesky(Cs[:3, :3])
    return L, float(np.sqrt(Cs[3, 3] / 4))


_NC = None


def _build_nc():
    import concourse.bacc as bacc
    import concourse.mybir as mybir
    import concourse.tile as tile

    L, rd = _weights()
    r_g = float(L[0, 0] / L[1, 0])   # g   = r_g*s1 + s2
    r_t1 = float(L[1, 0] / L[2, 0])  # t1' = r_t1*g + s3 = t1/L20
    r_t2 = float(L[1, 1] / L[2, 1])  # t2' = r_t2*s2 + s3 = t2/L21
    sc1, sc2, sc3 = float(L[2, 0]), float(L[2, 1]), float(L[2, 2])

    f32 = mybir.dt.float32
    bf16 = mybir.dt.bfloat16
    Sq = mybir.ActivationFunctionType.Square
    ALU = mybir.AluOpType

    nc = bacc.Bacc()
    # host packs each core's shard chunk-major, bf16: for each chunk,
    # contiguous [P, F] blocks [u0 v1 w2 | u1 v0 u2 w0 w1 v2 | sd]
    packed = nc.dram_tensor("packed", [P, 10 * J], bf16, kind="ExternalInput")
    out = nc.dram_tensor("out", [P, 2 * NT], f32, kind="ExternalOutput")

    with tile.TileContext(nc) as tc:
        with (
            tc.tile_pool(name="io", bufs=4) as io,
            tc.tile_pool(name="mid", bufs=3) as mid,
            tc.tile_pool(name="stats", bufs=1) as stats_pool,
        ):
            stats = stats_pool.tile([P, 2 * NT], f32)

            c0 = 0
            for t, F in enumerate(CHUNKS):
                buf = io.tile([P, 10 * F], bf16, tag="buf")
                nc.sync.dma_start(out=buf[:], in_=packed[:, c0:c0 + 10 * F])
                c0 += 10 * F

                u0 = buf[:, 0 * F:1 * F]
                v1 = buf[:, 1 * F:2 * F]
                w2 = buf[:, 2 * F:3 * F]
                u1, v0 = buf[:, 3 * F:4 * F], buf[:, 4 * F:5 * F]
                u2, w0 = buf[:, 5 * F:6 * F], buf[:, 6 * F:7 * F]
                w1, v2 = buf[:, 7 * F:8 * F], buf[:, 8 * F:9 * F]
                sd = buf[:, 9 * F:10 * F]

                # shear sums h4|h5|h6 in ONE strided tensor_tensor: the
                # packed layout puts the pairs adjacent ([u1 v0][u2 w0]
                # [w1 v2]), so in0 = blocks 3,5,7 and in1 = blocks 4,6,8
                # as 3D APs with outer stride 2F (inner step 1 keeps 2x)
                S = mid.tile([P, 3 * F], bf16, tag="S")
                pairs = buf[:, 3 * F:9 * F].rearrange(
                    "p (k two f) -> p k two f", k=3, two=2, f=F)
                nc.vector.tensor_add(
                    S[:].rearrange("p (k f) -> p k f", k=3, f=F),
                    pairs[:, :, 0, :], pairs[:, :, 1, :])

                # Cholesky rotation on DVE
                g = mid.tile([P, F], bf16, tag="g")
                t1 = mid.tile([P, F], bf16, tag="t1")
                t2 = mid.tile([P, F], bf16, tag="t2")
                if USE_STT:
                    # fused (in0*k) + in1 on DVE for the chained t1 path;
                    # t2's scaled copy rides on ACT (which has slack) so
                    # its add runs at 2x on DVE instead of a 1x stt
                    nc.vector.scalar_tensor_tensor(
                        out=g, in0=u0, scalar=r_g, in1=v1,
                        op0=ALU.mult, op1=ALU.add)
                    nc.vector.scalar_tensor_tensor(
                        out=t1, in0=g, scalar=r_t1, in1=w2,
                        op0=ALU.mult, op1=ALU.add)
                    pt2 = mid.tile([P, F], bf16, tag="pt2")
                    nc.scalar.mul(pt2, v1, r_t2)
                    nc.vector.tensor_add(t2, pt2, w2)
                else:
                    # scaled copy (4x tensor_scalar) + add (2x tensor_tensor)
                    pg = mid.tile([P, F], bf16, tag="pg")
                    nc.vector.tensor_scalar_mul(pg, u0, r_g)
                    nc.vector.tensor_add(g, pg, v1)
                    pt1 = mid.tile([P, F], bf16, tag="pt1")
                    nc.vector.tensor_scalar_mul(pt1, g, r_t1)
                    nc.vector.tensor_add(t1, pt1, w2)
                    pt2 = mid.tile([P, F], bf16, tag="pt2")
                    nc.vector.tensor_scalar_mul(pt2, v1, r_t2)
                    nc.vector.tensor_add(t2, pt2, w2)

                # mask (bf16 0/1) + per-chunk count via fused row-sum accum
                m = mid.tile([P, F], bf16, tag="m")
                nc.vector.tensor_scalar(
                    out=m, in0=sd, scalar1=THRESH, scalar2=None,
                    op0=ALU.is_lt, op1=ALU.add,
                    accum_out=stats[:, NT + t:NT + t + 1])

                # all 6 weighted squares on ACT (scale applied before Square)
                q = mid.tile([P, F], bf16, tag="q")
                if USE_REDUCE:
                    # squares written POINT-INTERLEAVED (strided ACT out
                    # APs), then ONE sequential-read tensor_reduce folds
                    # all 6 terms per point
                    Z = mid.tile([P, 6 * F], bf16, tag="Z")
                    Zi = Z[:].rearrange("p (f k) -> p f k", f=F, k=6)
                    nc.scalar.activation(Zi[:, :, 0], t1, Sq, scale=sc1)
                    nc.scalar.activation(Zi[:, :, 1], t2, Sq, scale=sc2)
                    nc.scalar.activation(Zi[:, :, 2], w2, Sq, scale=sc3)
                    nc.scalar.activation(
                        Zi[:, :, 3:6].rearrange("p f b -> p b f"),
                        S[:].rearrange("p (b f) -> p b f", b=3, f=F),
                        Sq, scale=rd)
                    with nc.allow_low_precision("bf16 6-term fold == "
                                                "TT-fold precision"):
                        nc.vector.tensor_reduce(
                            out=q[:], in_=Zi, axis=mybir.AxisListType.X,
                            op=ALU.add)
                else:
                    ZA = mid.tile([P, 3 * F], bf16, tag="ZA")
                    nc.scalar.activation(ZA[:, 0:F], t1, Sq, scale=sc1)
                    nc.scalar.activation(ZA[:, F:2 * F], t2, Sq, scale=sc2)
                    nc.scalar.activation(ZA[:, 2 * F:3 * F], w2, Sq, scale=sc3)
                    ZB = mid.tile([P, 3 * F], bf16, tag="ZB")
                    nc.scalar.activation(ZB, S, Sq, scale=rd)

                    # fold 6 -> 1 (wide + 2 narrow adds on DVE); the wide
                    # add runs in place over ZA to save an SBUF tile
                    nc.vector.tensor_add(ZA, ZA, ZB)
                    x = mid.tile([P, F], bf16, tag="x")
                    nc.vector.tensor_add(x, ZA[:, 0:F], ZA[:, F:2 * F])
                    nc.vector.tensor_add(q, x, ZA[:, 2 * F:3 * F])

                # qm = mask * q (in place over m), then ssq partial
                # sum(q^2 m) with f32 accum
                qm = m
                nc.vector.tensor_mul(qm, q, m)
                if USE_TTR:
                    # fused mult + row-accum on DVE (out = stride-0 dummy)
                    junk = mid.tile([P, 1], bf16, tag="junk")
                    nc.vector.tensor_tensor_reduce(
                        out=junk.broadcast_to((P, F)), in0=qm, in1=q,
                        scale=1.0, scalar=0.0,
                        op0=ALU.mult, op1=ALU.add,
                        accum_out=stats[:, t:t + 1])
                else:
                    # Square(qm) = q^2 m^2 = q^2 m, accum on ACT
                    junk = mid.tile([P, F], bf16, tag="junk")
                    nc.scalar.activation(
                        junk, qm, Sq, accum_out=stats[:, t:t + 1])

            nc.sync.dma_start(out=out[:, :], in_=stats[:])

    nc.compile()
    return nc


def _get_nc():
    global _NC
    if _NC is None:
        _NC = _build_nc()
    return _NC


def _run(in_maps, trace=False, **kwargs):
    from concourse.bass_utils import run_bass_kernel_spmd

    nc = _get_nc()
    return run_bass_kernel_spmd(
        nc, in_maps, core_ids=list(range(NCORES)), trace=trace, **kwargs)


def _make_in_maps(grad_u, grad_v, grad_w, gt_sdf):
    import ml_dtypes

    bf16 = ml_dtypes.bfloat16
    grad_u = np.asarray(grad_u, dtype=np.float32).astype(bf16)
    grad_v = np.asarray(grad_v, dtype=np.float32).astype(bf16)
    grad_w = np.asarray(grad_w, dtype=np.float32).astype(bf16)
    gt_sdf = np.asarray(gt_sdf, dtype=np.float32).astype(bf16)
    in_maps = []
    for c in range(NCORES):
        sl = slice(c * N_LOCAL, (c + 1) * N_LOCAL)
        gu = grad_u[sl].reshape(P, J, 3)
        gv = grad_v[sl].reshape(P, J, 3)
        gw = grad_w[sl].reshape(P, J, 3)
        sd = gt_sdf[sl].reshape(P, J)
        parts = []
        off = 0
        for F in CHUNKS:
            s = slice(off, off + F)
            parts += [gu[:, s, 0], gv[:, s, 1], gw[:, s, 2],
                      gu[:, s, 1], gv[:, s, 0],
                      gu[:, s, 2], gw[:, s, 0],
                      gw[:, s, 1], gv[:, s, 2],
                      sd[:, s]]
            off += F
        packed = np.ascontiguousarray(np.concatenate(parts, axis=1))
        in_maps.append({"packed": packed})
    return in_maps


def _finalize(results):
    ssq = 0.0
    cnt = 0.0
    for res in results:
        st = np.asarray(res["out"], dtype=np.float64)
        ssq += st[:, :NT].sum()
        cnt += st[:, NT:].sum()
    Wv = np.sqrt(ssq)
    return np.float32(Wv / cnt)


def kernel(grad_u, grad_v, grad_w, gt_sdf):
    in_maps = _make_in_maps(grad_u, grad_v, grad_w, gt_sdf)
    res = _run(in_maps, trace=False)
    return _finalize(res.results)
